# revision 1
# baseline (speedup 1.0000x reference)
"""Trainium2 Bass kernel for a full GPT block (LN -> QKV+RoPE -> full attention
-> out-proj -> residual -> LN -> GELU MLP -> residual).

Sharding: 8 cores = (batch b = core//2) x (query-half = core%2). Each core
redundantly computes K/V for its batch's full 2048 tokens (zero collectives);
Q/out-proj/MLP only for its own 1024 tokens. All per-core variation is shipped
as data (same SPMD program on every core).

Layout convention on device: activations are kept transposed [d_model on
partitions, tokens on free] so TensorE matmuls (which contract over the
partition axis) can consume them directly; LayerNorm stats for LN1 are taken in
natural [t, d] orientation before transposing via PE; LN2 stats use
ones-matmuls over the partition axis plus a PE broadcast.
"""

import os
import sys

sys.path.insert(0, "/opt/trn_rl_repo")

from contextlib import ExitStack

import ml_dtypes
import numpy as np

import concourse.bass as bass
from concourse import bacc
import concourse.mybir as mybir
import concourse.tile as tile
from concourse.bass_utils import run_bass_kernel_spmd
from concourse.masks import make_identity

B, T, D = 4, 2048, 512
H = 8
HD = 64
HALF = 32
EPS = 1e-5
TQ = T // 2          # tokens per core (query half)
DFF = 4 * D          # 2048
NC = 8

F32 = mybir.dt.float32
BF16 = mybir.dt.bfloat16
AF = mybir.ActivationFunctionType
ALU = mybir.AluOpType

_CACHE = {}


def _rope_tables():
    inv_freq = 1.0 / (10000.0 ** (np.arange(HALF, dtype=np.float64) / HALF))
    angle = np.arange(T, dtype=np.float64)[:, None] * inv_freq[None, :]
    cos = np.cos(angle).astype(np.float32)   # (T, 32)
    sin = np.sin(angle).astype(np.float32)
    # replicated to the 128-partition layout of q^T/k^T tiles:
    # rows [0:32]=head_even half1, [32:64]=head_even half2, [64:96]=head_odd h1,
    # [96:128]=head_odd h2. cos repeats every 32 rows; the swap-multiplier sign
    # is -sin for half1 rows and +sin for half2 rows.
    crep = np.tile(cos.T, (4, 1))                                  # (128, T)
    srep = np.concatenate([-sin.T, sin.T, -sin.T, sin.T], axis=0)  # (128, T)
    return crep.astype(ml_dtypes.bfloat16), srep.astype(ml_dtypes.bfloat16)


def _build_program():
    nc = bacc.Bacc("TRN2", target_bir_lowering=False)
    dp = nc.declare_dram_parameter
    d_x = dp("xb", [T, D], F32, isOutput=False)        # full batch, for K/V
    d_xq = dp("xq", [TQ, D], F32, isOutput=False)      # this core's half, for Q
    d_xt = dp("xt", [D, TQ], F32, isOutput=False)      # x^T half, residual
    d_wqkv = dp("wqkv", [D, 3 * D], BF16, isOutput=False)
    d_wout = dp("wout", [D, D], BF16, isOutput=False)
    d_w1 = dp("w1", [D, DFF], BF16, isOutput=False)
    d_w2 = dp("w2", [DFF, D], BF16, isOutput=False)
    d_b1 = dp("b1t", [128, DFF // 128], F32, isOutput=False)
    d_b2 = dp("b2t", [128, D // 128], F32, isOutput=False)
    d_ck = dp("cos_k", [128, T], BF16, isOutput=False)
    d_sk = dp("sin_k", [128, T], BF16, isOutput=False)
    d_cq = dp("cos_q", [128, TQ], BF16, isOutput=False)
    d_sq = dp("sin_q", [128, TQ], BF16, isOutput=False)
    d_out = dp("out", [TQ, D], F32, isOutput=True)

    ctx = ExitStack()
    with ctx:
        tc = ctx.enter_context(tile.TileContext(nc))
        # ---------------- persistent pools ----------------
        pw = ctx.enter_context(tc.tile_pool(name="weights", bufs=1))
        pact = ctx.enter_context(tc.tile_pool(name="acts", bufs=1))
        pconst = ctx.enter_context(tc.tile_pool(name="consts", bufs=1))
        psmall = ctx.enter_context(tc.tile_pool(name="small", bufs=2))
        pstat = ctx.enter_context(tc.tile_pool(name="stat", bufs=1))

        wqkv = pw.tile([128, 4, 3 * D], BF16)      # [dchunk] x j
        wout = pw.tile([128, 4, D], BF16)
        w1 = pw.tile([128, 4, DFF], BF16)
        w2 = pw.tile([128, 16, D], BF16)
        b1t = pw.tile([128, DFF // 128], F32)
        b2t = pw.tile([128, D // 128], F32)
        for c in range(4):
            nc.sync.dma_start(wqkv[:, c], d_wqkv[c * 128:(c + 1) * 128, :])
            nc.sync.dma_start(wout[:, c], d_wout[c * 128:(c + 1) * 128, :])
            nc.sync.dma_start(w1[:, c], d_w1[c * 128:(c + 1) * 128, :])
        for c in range(16):
            nc.sync.dma_start(w2[:, c], d_w2[c * 128:(c + 1) * 128, :])
        nc.sync.dma_start(b1t[:], d_b1[:, :])
        nc.sync.dma_start(b2t[:], d_b2[:, :])

        cos_k = pconst.tile([128, T], BF16)
        sin_k = pconst.tile([128, T], BF16)
        cos_q = pconst.tile([128, TQ], BF16)
        sin_q = pconst.tile([128, TQ], BF16)
        nc.sync.dma_start(cos_k[:], d_ck[:])
        nc.sync.dma_start(sin_k[:], d_sk[:])
        nc.sync.dma_start(cos_q[:], d_cq[:])
        nc.sync.dma_start(sin_q[:], d_sq[:])

        ident_bf = pconst.tile([128, 128], BF16)
        ident_f32 = pconst.tile([128, 128], F32)
        make_identity(nc, ident_bf)
        make_identity(nc, ident_f32)
        ones_bf = pconst.tile([128, 128], BF16)
        nc.gpsimd.memset(ones_bf[:], 1.0)
        ones_f32 = pconst.tile([128, 128], F32)
        nc.gpsimd.memset(ones_f32[:], 1.0)
        eps_t = pconst.tile([128, 1], F32)
        nc.gpsimd.memset(eps_t[:], EPS)

        xt = pact.tile([128, 4, TQ], F32)          # x^T (residual source)
        for c in range(4):
            nc.sync.dma_start(xt[:, c], d_xt[c * 128:(c + 1) * 128, :])

        # persistent activations
        kT = pact.tile([128, 4, T], BF16)          # k^T (RoPE'd)
        qT = pact.tile([128, 4, TQ], BF16)         # q^T (RoPE'd)
        # v natural, per-head blocks of 65 cols: 64 v dims + a trailing ones
        # column so the PV matmul also produces the softmax rowsum (row 64)
        vnat = pact.tile([128, 16, 8, 65], BF16)
        nc.gpsimd.memset(vnat[:, :, :, 64:65], 1.0)
        attnT = pact.tile([128, 4, TQ], BF16)      # normalized attn out ^T
        r1 = pact.tile([128, 4, TQ], F32)          # x + attn_out  (^T)

        def layernorm_tile(pool, x_tile, n_bf):
            """x_tile [128,512] f32 -> n_bf [128,512] bf16 (normalized)."""
            s1 = psmall.tile([128, 1], F32, tag="ln_s1")
            mu = psmall.tile([128, 1], F32, tag="ln_mu")
            m2 = psmall.tile([128, 1], F32, tag="ln_m2")
            var = psmall.tile([128, 1], F32, tag="ln_var")
            std = psmall.tile([128, 1], F32, tag="ln_std")
            rstd = psmall.tile([128, 1], F32, tag="ln_rstd")
            nmr = psmall.tile([128, 1], F32, tag="ln_nmr")
            sqs = pool.tile([128, D], BF16, tag="ln_sq_scratch")
            s2 = psmall.tile([128, 1], F32, tag="ln_s2")
            nc.vector.tensor_reduce(s1[:], x_tile[:], mybir.AxisListType.X, ALU.add)
            nc.scalar.activation(sqs[:], x_tile[:], AF.Square, accum_out=s2[:])
            nc.vector.tensor_scalar_mul(mu[:], s1[:], 1.0 / D)
            nc.vector.tensor_mul(m2[:], mu[:], mu[:])
            nc.vector.scalar_tensor_tensor(
                var[:], s2[:], 1.0 / D, m2[:], op0=ALU.mult, op1=ALU.subtract)
            nc.scalar.activation(std[:], var[:], AF.Sqrt, bias=eps_t[:], scale=1.0)
            nc.vector.reciprocal(rstd[:], std[:])
            nc.vector.scalar_tensor_tensor(
                nmr[:], mu[:], -1.0, rstd[:], op0=ALU.mult, op1=ALU.mult)
            nc.scalar.activation(n_bf[:], x_tile[:], AF.Identity,
                                 bias=nmr[:], scale=rstd[:])

        # ============ phase A: LN1 + transposes (K/V path, full T) ============
        with tc.tile_pool(name="phA", bufs=2) as pa, \
             tc.tile_pool(name="phA1", bufs=1) as pa1, \
             tc.tile_pool(name="phArope", bufs=2) as pr, \
             tc.tile_pool(name="psumA", bufs=2, space="PSUM") as psum:
            nT = pa1.tile([128, 4, T], BF16, tag="nT")       # LN1(x)^T full
            nTq = pa1.tile([128, 4, TQ], BF16, tag="nTq")    # LN1(x)^T q half
            for i in range(T // 128):
                x_t = pa.tile([128, D], F32, tag="x_in")
                nc.sync.dma_start(x_t[:], d_x[i * 128:(i + 1) * 128, :])
                n_bf = pa.tile([128, D], BF16, tag="n_bf")
                layernorm_tile(pa, x_t, n_bf)
                for c in range(4):
                    pst = psum.tile([128, 128], BF16, tag="ps_tr")
                    nc.tensor.transpose(pst[:], n_bf[:, c * 128:(c + 1) * 128],
                                        ident_bf[:])
                    nc.vector.tensor_copy(nT[:, c, i * 128:(i + 1) * 128], pst[:])
            for i in range(TQ // 128):
                x_t = pa.tile([128, D], F32, tag="x_in")
                nc.sync.dma_start(x_t[:], d_xq[i * 128:(i + 1) * 128, :])
                n_bf = pa.tile([128, D], BF16, tag="n_bf")
                layernorm_tile(pa, x_t, n_bf)
                for c in range(4):
                    pst = psum.tile([128, 128], BF16, tag="ps_tr")
                    nc.tensor.transpose(pst[:], n_bf[:, c * 128:(c + 1) * 128],
                                        ident_bf[:])
                    nc.vector.tensor_copy(nTq[:, c, i * 128:(i + 1) * 128], pst[:])

            # ---------------- QKV matmuls ----------------
            # q^T [j,t] over the core's half
            for jb in range(4):
                for tcn in range(TQ // 512):
                    ps = psum.tile([128, 512], F32, tag="ps_mm")
                    for c in range(4):
                        nc.tensor.matmul(
                            ps[:], wqkv[:, c, jb * 128:(jb + 1) * 128],
                            nTq[:, c, tcn * 512:(tcn + 1) * 512],
                            start=(c == 0), stop=(c == 3))
                    nc.scalar.copy(qT[:, jb, tcn * 512:(tcn + 1) * 512], ps[:])
            # k^T [j,t] over full T
            for jb in range(4):
                for tcn in range(T // 512):
                    ps = psum.tile([128, 512], F32, tag="ps_mm")
                    for c in range(4):
                        nc.tensor.matmul(
                            ps[:], wqkv[:, c, 512 + jb * 128:512 + (jb + 1) * 128],
                            nT[:, c, tcn * 512:(tcn + 1) * 512],
                            start=(c == 0), stop=(c == 3))
                    nc.scalar.copy(kT[:, jb, tcn * 512:(tcn + 1) * 512], ps[:])
            # v natural [t, dv] over full T (lhsT = nT chunks)
            for tb in range(T // 128):
                ps = psum.tile([128, 8, 64], F32, tag="ps_mm2")
                for c in range(4):
                    nc.tensor.matmul(
                        ps[:], nT[:, c, tb * 128:(tb + 1) * 128],
                        wqkv[:, c, 1024:1536],
                        start=(c == 0), stop=(c == 3))
                nc.vector.tensor_copy(vnat[:, tb, :, 0:64], ps[:])

            # ---------------- RoPE on q^T and k^T ----------------
            def rope(tsb, jb, t0, cos_t, sin_t):
                W = 512
                view = tsb[:, jb, t0:t0 + W]
                cs = slice(t0, t0 + W)
                qsw = pr.tile([128, W], BF16, tag="rope_swap")
                nc.sync.dma_start(qsw[0:32], view[32:64])
                nc.sync.dma_start(qsw[32:64], view[0:32])
                nc.sync.dma_start(qsw[64:96], view[96:128])
                nc.sync.dma_start(qsw[96:128], view[64:96])
                m1 = pr.tile([128, W], BF16, tag="rope_m1")
                nc.vector.tensor_mul(m1[:], view, cos_t[:, cs])
                m2 = pr.tile([128, W], BF16, tag="rope_m2")
                nc.vector.tensor_mul(m2[:], qsw[:], sin_t[:, cs])
                nc.vector.tensor_add(view, m1[:], m2[:])

            for jb in range(4):
                for t0 in range(0, TQ, 512):
                    rope(qT, jb, t0, cos_q, sin_q)
                for t0 in range(0, T, 512):
                    rope(kT, jb, t0, cos_k, sin_k)

        # ============ phase B: attention ============
        with tc.tile_pool(name="phB", bufs=2) as pb, \
             tc.tile_pool(name="phBe", bufs=20) as pbe, \
             tc.tile_pool(name="psumB", bufs=2, space="PSUM") as psum:
            for h in range(H):
                g, par = h // 2, h % 2
                po = par * 64          # partition offset of this head's rows
                for qc in range(TQ // 512):
                    qs = slice(qc * 512, (qc + 1) * 512)
                    etiles = []
                    for kb in range(T // 128):
                        ps_s = psum.tile([128, 512], F32, tag="ps_score")
                        nc.tensor.matmul(
                            ps_s[:],
                            kT[po:po + 64, g, kb * 128:(kb + 1) * 128],
                            qT[po:po + 64, g, qs],
                            start=True, stop=True)
                        e_t = pbe.tile([128, 512], BF16, tag="E")
                        nc.scalar.activation(e_t[:], ps_s[:], AF.Exp, scale=0.125)
                        etiles.append(e_t)
                    # O^T (rows 0..63) plus softmax rowsum (row 64), base 0
                    pv = psum.tile([128, 512], F32, tag="ps_pv")
                    for kb in range(T // 128):
                        nc.tensor.matmul(
                            pv[0:65],
                            vnat[:, kb, h, :],
                            etiles[kb][:],
                            start=(kb == 0), stop=(kb == T // 128 - 1))
                    # normalize: attnT = O * bcast(1/rowsum)
                    rinv = pb.tile([128, 512], F32, tag="rinv")
                    nc.vector.reciprocal(rinv[64:65], pv[64:65])
                    rinv_bf = pb.tile([128, 512], BF16, tag="rinv_bf")
                    nc.vector.tensor_copy(rinv_bf[64:65], rinv[64:65])
                    bc = psum.tile([128, 512], F32, tag="ps_bc")
                    nc.tensor.matmul(
                        bc[0:64],
                        ones_bf[64:65, 0:64],
                        rinv_bf[64:65, :],
                        start=True, stop=True)
                    o_t = pb.tile([128, 512], BF16, tag="o_t")
                    nc.scalar.copy(o_t[0:64], pv[0:64])
                    # partition-shifted write is legal (out base 64, 64 rows)
                    nc.vector.tensor_mul(attnT[po:po + 64, g, qs],
                                         o_t[0:64], bc[0:64])

        # ============ phase C: out-proj + residual + LN2 stats ============
        with tc.tile_pool(name="phC", bufs=2) as pc, \
             tc.tile_pool(name="psumC", bufs=2, space="PSUM") as psum:
            for db in range(4):
                for qc in range(TQ // 512):
                    qs = slice(qc * 512, (qc + 1) * 512)
                    ps = psum.tile([128, 512], F32, tag="ps_mm")
                    for c in range(4):
                        nc.tensor.matmul(
                            ps[:], wout[:, c, db * 128:(db + 1) * 128],
                            attnT[:, c, qs],
                            start=(c == 0), stop=(c == 3))
                    nc.vector.tensor_add(r1[:, db, qs], ps[:], xt[:, db, qs])

            # LN2 stats over partition axis via ones-matmuls
            # (s_mu = mean; s_a ends as rstd; s_b is scratch)
            mus = pstat.tile([1, TQ], F32, tag="mus")
            sqs = pstat.tile([1, TQ], F32, tag="sqs")
            rstd2 = pstat.tile([1, TQ], F32, tag="rstd2")
            for qc in range(TQ // 512):
                qs = slice(qc * 512, (qc + 1) * 512)
                ps_mu = psum.tile([1, 512], F32, tag="ps_stat")
                for c in range(4):
                    nc.tensor.matmul(ps_mu[:], ones_f32[:, 0:1], r1[:, c, qs],
                                     start=(c == 0), stop=(c == 3))
                nc.vector.tensor_scalar(mus[:, qs], ps_mu[:], 1.0 / D, None,
                                        ALU.mult)
                sq_t = pc.tile([128, 4, 512], BF16, tag="sq_t")
                for c in range(4):
                    nc.vector.tensor_mul(sq_t[:, c], r1[:, c, qs], r1[:, c, qs])
                ps_sq = psum.tile([1, 512], F32, tag="ps_stat")
                for c in range(4):
                    nc.tensor.matmul(ps_sq[:], ones_bf[:, 0:1], sq_t[:, c],
                                     start=(c == 0), stop=(c == 3))
                nc.vector.tensor_scalar(sqs[:, qs], ps_sq[:], 1.0 / D, None,
                                        ALU.mult)
            # rstd2 = 1/sqrt(sqs - mus^2 + eps); reuse tiles to save SBUF
            tmp2 = pstat.tile([1, TQ], F32, tag="tmp2")
            nc.vector.tensor_mul(tmp2[:], mus[:], mus[:])        # mu^2
            nc.vector.tensor_sub(sqs[:], sqs[:], tmp2[:])        # var
            nc.scalar.activation(tmp2[:], sqs[:], AF.Sqrt, bias=eps_t[0:1], scale=1.0)
            nc.vector.reciprocal(rstd2[:], tmp2[:])

            # h2 = (r1 - mu)*rstd  via PE broadcast of the [1,TQ] stats
            h2 = pact.tile([128, 4, TQ], BF16)
            for qc in range(TQ // 512):
                qs = slice(qc * 512, (qc + 1) * 512)
                bmu = psum.tile([128, 512], F32, tag="ps_bc")
                nc.tensor.matmul(bmu[:], ones_f32[0:1, :], mus[:, qs],
                                 start=True, stop=True)
                brs = psum.tile([128, 512], F32, tag="ps_bc")
                nc.tensor.matmul(brs[:], ones_f32[0:1, :], rstd2[:, qs],
                                 start=True, stop=True)
                for c in range(4):
                    t_sub = pc.tile([128, 512], F32, tag="t_sub")
                    nc.vector.tensor_sub(t_sub[:], r1[:, c, qs], bmu[:])
                    nc.vector.tensor_mul(h2[:, c, qs], t_sub[:], brs[:])

        # ============ phase D: MLP ============
        with tc.tile_pool(name="phD", bufs=2) as pd, \
             tc.tile_pool(name="phDa", bufs=1) as pda, \
             tc.tile_pool(name="psumD", bufs=2, space="PSUM") as psum:
            for qc in range(TQ // 512):
                qs = slice(qc * 512, (qc + 1) * 512)
                a_t = pda.tile([128, 16, 512], BF16, tag="a_t")
                for jb in range(16):
                    ps = psum.tile([128, 512], F32, tag="ps_mm")
                    for c in range(4):
                        nc.tensor.matmul(
                            ps[:], w1[:, c, jb * 128:(jb + 1) * 128],
                            h2[:, c, qs], start=(c == 0), stop=(c == 3))
                    nc.scalar.activation(a_t[:, jb], ps[:], AF.Gelu,
                                         bias=b1t[:, jb:jb + 1], scale=1.0)
                for db in range(4):
                    ps = psum.tile([128, 512], F32, tag="ps_mm")
                    for jb in range(16):
                        nc.tensor.matmul(
                            ps[:], w2[:, jb, db * 128:(db + 1) * 128],
                            a_t[:, jb], start=(jb == 0), stop=(jb == 15))
                    t_b = pd.tile([128, 512], F32, tag="t_b")
                    nc.scalar.activation(t_b[:], ps[:], AF.Identity,
                                         bias=b2t[:, db:db + 1], scale=1.0)
                    outT = pd.tile([128, 512], F32, tag="outT")
                    nc.vector.tensor_add(outT[:], t_b[:], r1[:, db, qs])
                    # transpose back to natural [t, d] and DMA out
                    for tcc in range(4):
                        pst = psum.tile([128, 128], F32, tag="ps_tr")
                        nc.tensor.transpose(
                            pst[:], outT[:, tcc * 128:(tcc + 1) * 128],
                            ident_f32[:])
                        ot = pd.tile([128, 128], F32, tag="ot")
                        nc.vector.tensor_copy(ot[:], pst[:])
                        t0 = qc * 512 + tcc * 128
                        nc.sync.dma_start(
                            d_out[t0:t0 + 128, db * 128:(db + 1) * 128], ot[:])
    nc.finalize()
    return nc


def kernel(x, ln1_g, ln1_b, w_qkv, w_out, ln2_g, ln2_b, w1, b1, w2, b2):
    x = np.asarray(x, np.float32)
    ln1_g = np.asarray(ln1_g, np.float32); ln1_b = np.asarray(ln1_b, np.float32)
    ln2_g = np.asarray(ln2_g, np.float32); ln2_b = np.asarray(ln2_b, np.float32)
    w_qkv = np.asarray(w_qkv, np.float32); w_out = np.asarray(w_out, np.float32)
    w1 = np.asarray(w1, np.float32); b1 = np.asarray(b1, np.float32)
    w2 = np.asarray(w2, np.float32); b2 = np.asarray(b2, np.float32)

    assert not np.any(ln1_b), "nonzero ln1_b not supported by this kernel"

    # exact-math folds: LN affine params into the adjacent weight matrices
    wqkv_f = ln1_g[:, None] * w_qkv
    w1_f = ln2_g[:, None] * w1
    b1_f = b1 + ln2_b @ w1

    crep, srep = _rope_tables()

    bf = ml_dtypes.bfloat16
    common = {
        "wqkv": np.ascontiguousarray(wqkv_f.astype(bf)),
        "wout": np.ascontiguousarray(w_out.astype(bf)),
        "w1": np.ascontiguousarray(w1_f.astype(bf)),
        "w2": np.ascontiguousarray(w2.astype(bf)),
        "b1t": np.ascontiguousarray(b1_f.reshape(DFF // 128, 128).T.astype(np.float32)),
        "b2t": np.ascontiguousarray(b2.reshape(D // 128, 128).T.astype(np.float32)),
        "cos_k": np.ascontiguousarray(crep),
        "sin_k": np.ascontiguousarray(srep),
    }
    in_maps = []
    for c in range(NC):
        b, half = c // 2, c % 2
        t0 = half * TQ
        m = dict(common)
        m["xb"] = np.ascontiguousarray(x[b])
        m["xq"] = np.ascontiguousarray(x[b, t0:t0 + TQ])
        m["xt"] = np.ascontiguousarray(x[b, t0:t0 + TQ].T)
        m["cos_q"] = np.ascontiguousarray(crep[:, t0:t0 + TQ])
        m["sin_q"] = np.ascontiguousarray(srep[:, t0:t0 + TQ])
        in_maps.append(m)

    if "prog" not in _CACHE:
        _CACHE["prog"] = _build_program()
    nc = _CACHE["prog"]

    _CACHE["in_maps"] = in_maps
    res = run_bass_kernel_spmd(nc, in_maps, core_ids=list(range(NC)))
    out = np.empty((B, T, D), np.float32)
    for c in range(NC):
        b, half = c // 2, c % 2
        out[b, half * TQ:(half + 1) * TQ] = res.results[c]["out"]
    return out



# revision 10
# speedup vs baseline: 1.4270x; 1.4270x over previous
"""Trainium2 Bass kernel for a full GPT block (LN -> QKV+RoPE -> full attention
-> out-proj -> residual -> LN -> GELU MLP -> residual).

Sharding: 8 cores = (batch b = core//2) x (query-half = core%2). Each core
redundantly computes K/V for its batch's full 2048 tokens (zero collectives);
Q/out-proj/MLP only for its own 1024 tokens. Tokens are ROTATED per core on
the host so the core's query half is always tokens [0, TQ) — attention is
permutation-invariant over keys, and RoPE tables are rotated to match.

Layout: activations live transposed [d_model on partitions, tokens on free].
LayerNorm stats are computed with ones-matmuls over the partition axis
(bf16, full PE rate) and broadcast back via a rank-1 PE matmul — no PE
transposes anywhere. The final output is DMA'd transposed and fixed on host.

Precision: weights/activations bf16 except (a) q/k projections run fp8e4
DoubleRow (softmax normalization washes out score noise), (b) PV runs fp8
DoubleRow (exp output + V cast to fp8), (c) optionally fc1 fp8. PSUM is f32
throughout; residual stream is f32.
"""

import sys

sys.path.insert(0, "/opt/trn_rl_repo")

from contextlib import ExitStack

import ml_dtypes
import numpy as np

import concourse.bass as bass  # noqa: F401
from concourse import bacc
import concourse.mybir as mybir
import concourse.tile as tile
from concourse.bass_utils import run_bass_kernel_spmd

B, T, D = 4, 2048, 512
H = 8
HD = 64
HALF = 32
EPS = 1e-5
TQ = T // 2          # tokens per core (query half)
DFF = 4 * D          # 2048
NC = 8
WS = 16.0            # fp8 weight pre-scale (power of 2, exact)

F32 = mybir.dt.float32
BF16 = mybir.dt.bfloat16
F8 = mybir.dt.float8e4
AF = mybir.ActivationFunctionType
ALU = mybir.AluOpType
DR = mybir.MatmulPerfMode.DoubleRow

FC1_FP8 = True       # fc1 (h2 @ w1) in fp8 DoubleRow
FC2_FP8 = False      # fc2 (a @ w2) in fp8 DoubleRow
DEBUG = False        # add intermediate DRAM outputs for stage-by-stage check

_CACHE = {}


def _rope_tables():
    inv_freq = 1.0 / (10000.0 ** (np.arange(HALF, dtype=np.float64) / HALF))
    angle = np.arange(T, dtype=np.float64)[:, None] * inv_freq[None, :]
    cos = np.cos(angle).astype(np.float32)   # (T, 32)
    sin = np.sin(angle).astype(np.float32)
    # replicated to the 128-partition layout of q^T/k^T tiles:
    # rows [0:32]=head_even half1, [32:64]=head_even half2, [64:96]=head_odd
    # h1, [96:128]=head_odd h2. cos repeats every 32 rows; the swap-multiplier
    # sign is -sin for half1 rows and +sin for half2 rows.
    crep = np.tile(cos.T, (4, 1))                                  # (128, T)
    srep = np.concatenate([-sin.T, sin.T, -sin.T, sin.T], axis=0)  # (128, T)
    return crep.astype(ml_dtypes.bfloat16), srep.astype(ml_dtypes.bfloat16)


def _build_program():
    h2_dt = F8 if FC1_FP8 else BF16
    a_dt = F8 if FC2_FP8 else BF16
    w1_dt = F8 if FC1_FP8 else BF16
    w2_dt = F8 if FC2_FP8 else BF16

    nc = bacc.Bacc("TRN2", target_bir_lowering=False)
    dp = nc.declare_dram_parameter
    d_xt = dp("xt", [D, T], BF16, isOutput=False)       # x^T, rotated
    d_wqk = dp("wqk", [D, 2 * D], F8, isOutput=False)   # x WS, ln1_g folded
    d_wv = dp("wv", [D, D], BF16, isOutput=False)       # ln1_g folded
    d_wout = dp("wout", [D, D], BF16, isOutput=False)
    d_w1 = dp("w1", [D, DFF], w1_dt, isOutput=False)    # (x WS) ln2_g folded
    d_w2 = dp("w2", [DFF, D], w2_dt, isOutput=False)    # (x WS)
    d_b1 = dp("b1t", [128, DFF // 128], F32, isOutput=False)
    d_ck = dp("cos_k", [128, T], BF16, isOutput=False)  # rotated
    d_sk = dp("sin_k", [128, T], BF16, isOutput=False)
    d_out = dp("out", [D, TQ], F32, isOutput=True)      # transposed out
    if DEBUG:
        d_dbg = {
            "nT8": dp("dbg_nT8", [128, 4, T], F8, isOutput=True),
            "nTb": dp("dbg_nTb", [128, 4, T], BF16, isOutput=True),
            "qT": dp("dbg_qT", [128, 4, TQ], BF16, isOutput=True),
            "kT": dp("dbg_kT", [128, 4, T], BF16, isOutput=True),
            "vnat": dp("dbg_vnat", [128, 16, 8, 66], F8, isOutput=True),
            "attnT": dp("dbg_attnT", [128, 4, TQ], BF16, isOutput=True),
            "r1": dp("dbg_r1", [128, 4, TQ], F32, isOutput=True),
            "h2": dp("dbg_h2", [128, 4, TQ],
                     F8 if FC1_FP8 else BF16, isOutput=True),
        }

    ctx = ExitStack()
    with ctx:
        tc = ctx.enter_context(tile.TileContext(nc))
        # ---------------- persistent pools ----------------
        pw = ctx.enter_context(tc.tile_pool(name="weights", bufs=1))
        pact = ctx.enter_context(tc.tile_pool(name="acts", bufs=1))
        pconst = ctx.enter_context(tc.tile_pool(name="consts", bufs=1))

        xt = pact.tile([128, 4, T], BF16)
        for c in range(4):
            nc.sync.dma_start(xt[:, c], d_xt[c * 128:(c + 1) * 128, :])

        wqk = pw.tile([128, 4, 2 * D], F8)
        wv = pw.tile([128, 4, D], BF16)
        wout = pw.tile([128, 4, D], BF16)
        w1 = pw.tile([128, 4, DFF], w1_dt)
        w2 = pw.tile([128, 16, D], w2_dt)
        b1t = pw.tile([128, DFF // 128], F32)
        for c in range(4):
            nc.sync.dma_start(wqk[:, c], d_wqk[c * 128:(c + 1) * 128, :])
        cos_k = pconst.tile([128, T], BF16)
        sin_k = pconst.tile([128, T], BF16)
        nc.sync.dma_start(cos_k[:], d_ck[:])
        nc.sync.dma_start(sin_k[:], d_sk[:])
        for c in range(4):
            nc.sync.dma_start(wv[:, c], d_wv[c * 128:(c + 1) * 128, :])
            nc.sync.dma_start(wout[:, c], d_wout[c * 128:(c + 1) * 128, :])
            nc.sync.dma_start(w1[:, c], d_w1[c * 128:(c + 1) * 128, :])
        for c in range(16):
            nc.sync.dma_start(w2[:, c], d_w2[c * 128:(c + 1) * 128, :])
        nc.sync.dma_start(b1t[:], d_b1[:, :])

        ones_bf = pconst.tile([128, 128], BF16)
        nc.gpsimd.memset(ones_bf[:], 1.0)
        inv_d = pconst.tile([128, 1], BF16)
        nc.gpsimd.memset(inv_d[:], 1.0 / D)
        eps_t = pconst.tile([128, 1], F32)
        nc.gpsimd.memset(eps_t[:], EPS)

        # persistent activations
        nT8 = pact.tile([128, 4, T], F8)           # LN1(x)^T fp8 (q/k GEMMs)
        nTb = pact.tile([128, 4, T], BF16)         # LN1(x)^T bf16 (v GEMM)
        kT = pact.tile([128, 4, T], BF16)          # k^T (RoPE'd, x WS)
        qT = pact.tile([128, 4, TQ], BF16)         # q^T (RoPE'd, x WS)
        # v natural, per-head blocks of 65 cols: 64 v dims + a trailing ones
        # column so the PV matmul also produces the softmax rowsum (row 64)
        vnat = pact.tile([128, 16, 8, 66], F8)
        nc.gpsimd.memset(vnat[:, :, :, 64:65], 1.0)
        nc.gpsimd.memset(vnat[:, :, :, 65:66], 0.0)
        attnT = pact.tile([128, 4, TQ], BF16)      # normalized attn out ^T
        r1 = pact.tile([128, 4, TQ], F32)          # x + attn_out  (^T)
        r1b = pact.tile([128, 4, TQ], BF16)        # bf16 copy for LN2 stats
        h2 = pact.tile([128, 4, TQ], h2_dt)        # LN2 out

        def ln_transposed(src_bf, n_chunks, ps_stat, ps_bc, pool, apply_srcs,
                          outs):
            """LayerNorm over the partition (d_model) axis of a transposed
            activation. src_bf: [128, 4, n_chunks*512] bf16 tile for stats.
            apply_srcs: per-c list of tiles to read in the apply step (f32 or
            bf16); outs: list of (tile, ) destinations written as
            (apply_src - mu) * rstd."""
            for i in range(n_chunks):
                cs = slice(i * 512, (i + 1) * 512)
                ps_mu = ps_stat.tile([1, 512], F32, tag="ps_mu")
                for c in range(4):
                    nc.tensor.matmul(ps_mu[:], inv_d[:, 0:1], src_bf[:, c, cs],
                                     start=(c == 0), stop=(c == 3))
                sq = pool.tile([128, 4, 512], BF16, tag="ln_sq")
                for c in range(4):
                    nc.vector.tensor_mul(sq[:, c], src_bf[:, c, cs],
                                         src_bf[:, c, cs])
                ps_sq = ps_stat.tile([1, 512], F32, tag="ps_sq")
                for c in range(4):
                    nc.tensor.matmul(ps_sq[:], inv_d[:, 0:1], sq[:, c],
                                     start=(c == 0), stop=(c == 3))
                mu_r = pool.tile([1, 512], BF16, tag="mu_r")
                nc.vector.tensor_copy(mu_r[:], ps_mu[:])
                ex2_r = pool.tile([1, 512], F32, tag="ex2_r")
                nc.vector.tensor_copy(ex2_r[:], ps_sq[:])
                # var = E[x^2] - mu^2 on the narrow stat rows
                m2_r = pool.tile([1, 512], F32, tag="m2_r")
                nc.vector.tensor_mul(m2_r[:], mu_r[:], mu_r[:])
                var_r = pool.tile([1, 512], BF16, tag="var_r")
                nc.vector.tensor_sub(var_r[:], ex2_r[:], m2_r[:])
                bc = ps_bc.tile([128, 2, 512], F32, tag="ps_bc")
                nc.tensor.matmul(bc[:, 0], ones_bf[0:1, :], mu_r[:],
                                 start=True, stop=True)
                nc.tensor.matmul(bc[:, 1], ones_bf[0:1, :], var_r[:],
                                 start=True, stop=True)
                std = pool.tile([128, 512], F32, tag="ln_std")
                nc.scalar.activation(std[:], bc[:, 1], AF.Sqrt, bias=eps_t[:],
                                     scale=1.0)
                brs = pool.tile([128, 512], F32, tag="ln_brs")
                nc.vector.reciprocal(brs[:], std[:])
                for c in range(4):
                    t_sub = pool.tile([128, 512], BF16, tag="ln_tsub")
                    nc.vector.tensor_sub(t_sub[:], apply_srcs[c][:, cs],
                                         bc[:, 0])
                    for out_t in outs:
                        nc.vector.tensor_mul(out_t[:, c, cs], t_sub[:], brs[:])

        # ============ phase A: LN1 + QKV + RoPE ============
        with tc.tile_pool(name="phA", bufs=2) as pa, \
             tc.tile_pool(name="phArope", bufs=2) as pr, \
             tc.tile_pool(name="psumSt", bufs=1, space="PSUM") as ps_stat, \
             tc.tile_pool(name="psumBc", bufs=2, space="PSUM") as ps_bc, \
             tc.tile_pool(name="psumA", bufs=2, space="PSUM") as psum:
            ln_transposed(xt, T // 512, ps_stat, ps_bc, pa,
                          [xt[:, c] for c in range(4)], [nT8, nTb])

            # q^T [j,t] over the core's half (fp8 DoubleRow)
            for jb in range(4):
                for tcn in range(TQ // 512):
                    ps = psum.tile([128, 512], F32, tag="ps_mm")
                    for c2 in range(2):
                        nc.tensor.matmul(
                            ps[:],
                            wqk[:, 2 * c2:2 * c2 + 2, jb * 128:(jb + 1) * 128],
                            nT8[:, 2 * c2:2 * c2 + 2, tcn * 512:(tcn + 1) * 512],
                            start=(c2 == 0), stop=(c2 == 1), perf_mode=DR)
                    nc.scalar.copy(qT[:, jb, tcn * 512:(tcn + 1) * 512], ps[:])
            # k^T [j,t] over full T (fp8 DoubleRow)
            for jb in range(4):
                for tcn in range(T // 512):
                    ps = psum.tile([128, 512], F32, tag="ps_mm")
                    for c2 in range(2):
                        nc.tensor.matmul(
                            ps[:],
                            wqk[:, 2 * c2:2 * c2 + 2,
                                512 + jb * 128:512 + (jb + 1) * 128],
                            nT8[:, 2 * c2:2 * c2 + 2, tcn * 512:(tcn + 1) * 512],
                            start=(c2 == 0), stop=(c2 == 1), perf_mode=DR)
                    nc.scalar.copy(kT[:, jb, tcn * 512:(tcn + 1) * 512], ps[:])
            # v natural [t, dv] over full T (bf16; lhsT = nTb chunks)
            for tb in range(T // 128):
                ps = psum.tile([128, 8, 64], F32, tag="ps_mm")
                for c in range(4):
                    nc.tensor.matmul(
                        ps[:], nTb[:, c, tb * 128:(tb + 1) * 128],
                        wv[:, c, :], start=(c == 0), stop=(c == 3))
                nc.scalar.copy(vnat[:, tb, :, 0:64], ps[:])

            # ---------------- RoPE on q^T and k^T ----------------
            def rope(tsb, jb, t0, cos_t, sin_t, coff):
                W = 512
                view = tsb[:, jb, t0:t0 + W]
                cs = slice(coff + t0, coff + t0 + W)
                qsw = pr.tile([128, W], BF16, tag="rope_swap")
                nc.sync.dma_start(qsw[0:32], view[32:64])
                nc.sync.dma_start(qsw[32:64], view[0:32])
                nc.sync.dma_start(qsw[64:96], view[96:128])
                nc.sync.dma_start(qsw[96:128], view[64:96])
                m1 = pr.tile([128, W], BF16, tag="rope_m1")
                nc.vector.tensor_mul(m1[:], view, cos_t[:, cs])
                m2 = pr.tile([128, W], BF16, tag="rope_m2")
                nc.vector.tensor_mul(m2[:], qsw[:], sin_t[:, cs])
                nc.vector.tensor_add(view, m1[:], m2[:])

            for jb in range(4):
                for t0 in range(0, TQ, 512):
                    rope(qT, jb, t0, cos_k, sin_k, 0)
                for t0 in range(0, T, 512):
                    rope(kT, jb, t0, cos_k, sin_k, 0)

        # ============ phase B: attention ============
        with tc.tile_pool(name="phB", bufs=2) as pb, \
             tc.tile_pool(name="phBe", bufs=4) as pbe, \
             tc.tile_pool(name="psumS", bufs=2, space="PSUM") as ps_s, \
             tc.tile_pool(name="psumPV", bufs=2, space="PSUM") as ps_pv, \
             tc.tile_pool(name="psumBc2", bufs=2, space="PSUM") as ps_bc2:
            for h in range(H):
                g, par = h // 2, h % 2
                po = par * 64          # partition offset of this head's rows
                for qc in range(TQ // 512):
                    qs = slice(qc * 512, (qc + 1) * 512)
                    pv = ps_pv.tile([128, 512], F32, tag="ps_pv")
                    for pair in range(T // 256):
                        ps2 = ps_s.tile([128, 2, 512], F32, tag="ps_score")
                        for hf in range(2):
                            kb = 2 * pair + hf
                            nc.tensor.matmul(
                                ps2[:, hf],
                                kT[po:po + 64, g, kb * 128:(kb + 1) * 128],
                                qT[po:po + 64, g, qs],
                                start=True, stop=True)
                        e2 = pbe.tile([128, 2, 512], F8, tag="E2")
                        nc.scalar.activation(e2[:], ps2[:], AF.Exp,
                                             scale=0.125 / (WS * WS))
                        # O^T (rows 0..63) + softmax rowsum (row 64)
                        nc.tensor.matmul(
                            pv[0:66],
                            vnat[:, 2 * pair:2 * pair + 2, h, :],
                            e2[:],
                            start=(pair == 0), stop=(pair == T // 256 - 1),
                            perf_mode=DR)
                    # normalize: attnT = O * bcast(1/rowsum)
                    rinv = pb.tile([128, 512], F32, tag="rinv")
                    nc.vector.reciprocal(rinv[64:65], pv[64:65])
                    rinv_bf = pb.tile([128, 512], BF16, tag="rinv_bf")
                    nc.vector.tensor_copy(rinv_bf[64:65], rinv[64:65])
                    bc = ps_bc2.tile([128, 512], F32, tag="ps_bc")
                    nc.tensor.matmul(
                        bc[0:64],
                        ones_bf[64:65, 0:64],
                        rinv_bf[64:65, :],
                        start=True, stop=True)
                    o_t = pb.tile([128, 512], BF16, tag="o_t")
                    nc.scalar.copy(o_t[0:64], pv[0:64])
                    # partition-shifted write is legal (out base 64, 64 rows)
                    nc.vector.tensor_mul(attnT[po:po + 64, g, qs],
                                         o_t[0:64], bc[0:64])

        # ============ phase C: out-proj + residual + LN2 ============
        with tc.tile_pool(name="phC", bufs=2) as pc, \
             tc.tile_pool(name="psumSt2", bufs=1, space="PSUM") as ps_stat2, \
             tc.tile_pool(name="psumBc3", bufs=2, space="PSUM") as ps_bc3, \
             tc.tile_pool(name="psumC", bufs=2, space="PSUM") as psum:
            for db in range(4):
                for qc in range(TQ // 512):
                    qs = slice(qc * 512, (qc + 1) * 512)
                    ps = psum.tile([128, 512], F32, tag="ps_mm")
                    for c in range(4):
                        nc.tensor.matmul(
                            ps[:], wout[:, c, db * 128:(db + 1) * 128],
                            attnT[:, c, qs],
                            start=(c == 0), stop=(c == 3))
                    nc.vector.tensor_add(r1[:, db, qs], ps[:], xt[:, db, qs])
                    nc.vector.tensor_copy(r1b[:, db, qs], r1[:, db, qs])

            ln_transposed(r1b, TQ // 512, ps_stat2, ps_bc3, pc,
                          [r1[:, c] for c in range(4)], [h2])

        # ============ phase D: MLP ============
        with tc.tile_pool(name="phD", bufs=2) as pd, \
             tc.tile_pool(name="phDa", bufs=1) as pda, \
             tc.tile_pool(name="psumD", bufs=2, space="PSUM") as psum:
            gelu_scale = 1.0 / WS if FC1_FP8 else 1.0
            fc2_scale = 1.0 / WS if FC2_FP8 else 1.0
            for qc in range(TQ // 512):
                qs = slice(qc * 512, (qc + 1) * 512)
                a_t = pda.tile([128, 16, 512], a_dt, tag="a_t")
                for jb in range(16):
                    ps = psum.tile([128, 512], F32, tag="ps_mm")
                    if FC1_FP8:
                        for c2 in range(2):
                            nc.tensor.matmul(
                                ps[:],
                                w1[:, 2 * c2:2 * c2 + 2,
                                   jb * 128:(jb + 1) * 128],
                                h2[:, 2 * c2:2 * c2 + 2, qs],
                                start=(c2 == 0), stop=(c2 == 1), perf_mode=DR)
                    else:
                        for c in range(4):
                            nc.tensor.matmul(
                                ps[:], w1[:, c, jb * 128:(jb + 1) * 128],
                                h2[:, c, qs], start=(c == 0), stop=(c == 3))
                    nc.scalar.activation(a_t[:, jb], ps[:], AF.Gelu,
                                         bias=b1t[:, jb:jb + 1],
                                         scale=gelu_scale)
                for db in range(4):
                    ps = psum.tile([128, 512], F32, tag="ps_mm")
                    if FC2_FP8:
                        for j2 in range(8):
                            nc.tensor.matmul(
                                ps[:],
                                w2[:, 2 * j2:2 * j2 + 2,
                                   db * 128:(db + 1) * 128],
                                a_t[:, 2 * j2:2 * j2 + 2],
                                start=(j2 == 0), stop=(j2 == 7), perf_mode=DR)
                    else:
                        for jb in range(16):
                            nc.tensor.matmul(
                                ps[:], w2[:, jb, db * 128:(db + 1) * 128],
                                a_t[:, jb], start=(jb == 0), stop=(jb == 15))
                    outT = pd.tile([128, 512], F32, tag="outT")
                    nc.vector.scalar_tensor_tensor(
                        outT[:], ps[:], fc2_scale, r1[:, db, qs],
                        op0=ALU.mult, op1=ALU.add)
                    nc.sync.dma_start(
                        d_out[db * 128:(db + 1) * 128, qs], outT[:])

        if DEBUG:
            for name, t in [("nT8", nT8), ("nTb", nTb), ("qT", qT),
                            ("kT", kT), ("vnat", vnat), ("attnT", attnT),
                            ("r1", r1), ("h2", h2)]:
                nc.sync.dma_start(d_dbg[name][:], t[:])
    nc.finalize()
    return nc


def kernel(x, ln1_g, ln1_b, w_qkv, w_out, ln2_g, ln2_b, w1, b1, w2, b2):
    x = np.asarray(x, np.float32)
    ln1_g = np.asarray(ln1_g, np.float32); ln1_b = np.asarray(ln1_b, np.float32)
    ln2_g = np.asarray(ln2_g, np.float32); ln2_b = np.asarray(ln2_b, np.float32)
    w_qkv = np.asarray(w_qkv, np.float32); w_out = np.asarray(w_out, np.float32)
    w1 = np.asarray(w1, np.float32); b1 = np.asarray(b1, np.float32)
    w2 = np.asarray(w2, np.float32); b2 = np.asarray(b2, np.float32)

    assert not np.any(ln1_b), "nonzero ln1_b not supported by this kernel"

    # exact-math folds: LN affine params into the adjacent weight matrices
    wqkv_f = ln1_g[:, None] * w_qkv
    w1_f = ln2_g[:, None] * w1
    b1_f = b1 + ln2_b @ w1

    bf = ml_dtypes.bfloat16
    f8 = ml_dtypes.float8_e4m3

    def to_f8(a, scale):
        return np.ascontiguousarray(
            np.clip(a * scale, -240.0, 240.0).astype(f8))

    crep, srep = _rope_tables()

    common = {
        "wqk": to_f8(wqkv_f[:, :2 * D], WS),
        "wv": np.ascontiguousarray(wqkv_f[:, 2 * D:].astype(bf)),
        "wout": np.ascontiguousarray(w_out.astype(bf)),
        "w1": (to_f8(w1_f, WS) if FC1_FP8
               else np.ascontiguousarray(w1_f.astype(bf))),
        "w2": (to_f8(w2, WS) if FC2_FP8
               else np.ascontiguousarray(w2.astype(bf))),
        "b1t": np.ascontiguousarray(
            b1_f.reshape(DFF // 128, 128).T.astype(np.float32)),
    }
    in_maps = []
    for c in range(NC):
        b, half = c // 2, c % 2
        t0 = half * TQ
        rot = np.r_[t0:T, 0:t0]
        m = dict(common)
        m["xt"] = np.ascontiguousarray(x[b].T[:, rot].astype(bf))
        m["cos_k"] = np.ascontiguousarray(crep[:, rot])
        m["sin_k"] = np.ascontiguousarray(srep[:, rot])
        in_maps.append(m)

    if "prog" not in _CACHE:
        _CACHE["prog"] = _build_program()
    nc = _CACHE["prog"]

    _CACHE["in_maps"] = in_maps
    res = run_bass_kernel_spmd(nc, in_maps, core_ids=list(range(NC)))
    out = np.empty((B, T, D), np.float32)
    for c in range(NC):
        b, half = c // 2, c % 2
        out[b, half * TQ:(half + 1) * TQ] = res.results[c]["out"].T
    out += b2[None, None, :]
    return out


# revision 20
# speedup vs baseline: 1.5744x; 1.1033x over previous
"""Trainium2 Bass kernel for a full GPT block (LN -> QKV+RoPE -> full attention
-> out-proj -> residual -> LN -> GELU MLP -> residual).

Sharding: 8 cores = (batch b = core//2) x (query-half = core%2). Each core
redundantly computes K/V for its batch's full 2048 tokens (zero collectives);
Q/out-proj/MLP only for its own 1024 tokens. Tokens are ROTATED per core on
the host so the core's query half is always tokens [0, TQ) — attention is
permutation-invariant over keys, and RoPE tables are rotated to match.

Layout: activations live transposed [d_model on partitions, tokens on free].
LayerNorm stats are computed with ones-matmuls over the partition axis
(bf16, full PE rate) and broadcast back via a rank-1 PE matmul — no PE
transposes anywhere. The final output is DMA'd transposed and fixed on host.

Precision: weights/activations bf16 except (a) q/k projections run fp8e4
DoubleRow (softmax normalization washes out score noise), (b) PV runs fp8
DoubleRow (exp output + V cast to fp8), (c) optionally fc1 fp8. PSUM is f32
throughout; residual stream is f32.
"""

import sys

sys.path.insert(0, "/opt/trn_rl_repo")

from contextlib import ExitStack

import ml_dtypes
import numpy as np

import concourse.bass as bass  # noqa: F401
from concourse import bacc
import concourse.mybir as mybir
import concourse.tile as tile
from concourse.bass_utils import run_bass_kernel_spmd

B, T, D = 4, 2048, 512
H = 8
HD = 64
HALF = 32
EPS = 1e-5
TQ = T // 2          # tokens per core (query half)
DFF = 4 * D          # 2048
NC = 8
WS = 16.0            # fp8 weight pre-scale (power of 2, exact)

F32 = mybir.dt.float32
BF16 = mybir.dt.bfloat16
F8 = mybir.dt.float8e4
AF = mybir.ActivationFunctionType
ALU = mybir.AluOpType
DR = mybir.MatmulPerfMode.DoubleRow

FC1_FP8 = True       # fc1 (h2 @ w1) in fp8 DoubleRow
FC2_FP8 = True       # fc2 (a @ w2) in fp8 DoubleRow
V_FP8 = True         # v projection in fp8 DoubleRow (drops the bf16 LN copy)
DEBUG = False        # add intermediate DRAM outputs for stage-by-stage check

_CACHE = {}


def _rope_tables():
    inv_freq = 1.0 / (10000.0 ** (np.arange(HALF, dtype=np.float64) / HALF))
    angle = np.arange(T, dtype=np.float64)[:, None] * inv_freq[None, :]
    cos = np.cos(angle).astype(np.float32)   # (T, 32)
    sin = np.sin(angle).astype(np.float32)
    # replicated to the 128-partition layout of q^T/k^T tiles:
    # rows [0:32]=head_even half1, [32:64]=head_even half2, [64:96]=head_odd
    # h1, [96:128]=head_odd h2. cos repeats every 32 rows; the swap-multiplier
    # sign is -sin for half1 rows and +sin for half2 rows.
    crep = np.tile(cos.T, (4, 1))                                  # (128, T)
    srep = np.concatenate([-sin.T, sin.T, -sin.T, sin.T], axis=0)  # (128, T)
    return crep.astype(ml_dtypes.bfloat16), srep.astype(ml_dtypes.bfloat16)


def _build_program():
    h2_dt = F8 if FC1_FP8 else BF16
    a_dt = F8 if FC2_FP8 else BF16
    w1_dt = F8 if FC1_FP8 else BF16
    w2_dt = F8 if FC2_FP8 else BF16

    nc = bacc.Bacc("TRN2", target_bir_lowering=False)
    dp = nc.declare_dram_parameter
    d_xt = dp("xt", [D, T], BF16, isOutput=False)       # x^T, rotated
    d_wqk = dp("wqk", [D, 2 * D], F8, isOutput=False)   # x WS, ln1_g folded
    d_wv = dp("wv", [D, D], F8 if V_FP8 else BF16, isOutput=False)
    d_wout = dp("wout", [D, D], BF16, isOutput=False)
    d_w1 = dp("w1", [D, DFF], w1_dt, isOutput=False)    # (x WS) ln2_g folded
    d_w2 = dp("w2", [DFF, D], w2_dt, isOutput=False)    # (x WS)
    d_b1 = dp("b1t", [128, DFF // 128], F32, isOutput=False)
    d_ck = dp("cos_k", [128, T], BF16, isOutput=False)  # rotated
    d_sk = dp("sin_k", [128, T], BF16, isOutput=False)
    d_out = dp("out", [D, TQ], F32, isOutput=True)      # transposed out
    if DEBUG:
        d_dbg = {
            "nT8": dp("dbg_nT8", [128, 4, T], F8, isOutput=True),
            "nTb": dp("dbg_nTb", [128, 4, T], BF16, isOutput=True),
            "qT": dp("dbg_qT", [128, 4, TQ], BF16, isOutput=True),
            "kT": dp("dbg_kT", [128, 4, T], BF16, isOutput=True),
            "vnat": dp("dbg_vnat", [128, 16, 8, 66], F8, isOutput=True),
            "attnT": dp("dbg_attnT", [128, 4, TQ], BF16, isOutput=True),
            "r1": dp("dbg_r1", [128, 4, TQ], F32, isOutput=True),
            "h2": dp("dbg_h2", [128, 4, TQ],
                     F8 if FC1_FP8 else BF16, isOutput=True),
        }

    ctx = ExitStack()
    with ctx:
        tc = ctx.enter_context(tile.TileContext(nc))
        # ---------------- persistent pools ----------------
        pw = ctx.enter_context(tc.tile_pool(name="weights", bufs=1))
        pact = ctx.enter_context(tc.tile_pool(name="acts", bufs=1))
        pconst = ctx.enter_context(tc.tile_pool(name="consts", bufs=1))

        xt = pact.tile([128, 4, T], BF16)
        for c in range(4):
            nc.sync.dma_start(xt[:, c], d_xt[c * 128:(c + 1) * 128, :])

        wqk = pw.tile([128, 4, 2 * D], F8)
        wv = pw.tile([128, 4, D], F8 if V_FP8 else BF16)
        wout = pw.tile([128, 4, D], BF16)
        w1 = pw.tile([128, 4, DFF], w1_dt)
        w2 = pw.tile([128, 16, D], w2_dt)
        b1t = pw.tile([128, DFF // 128], F32)
        for c in range(4):
            nc.sync.dma_start(wqk[:, c], d_wqk[c * 128:(c + 1) * 128, :])
        cos_k = pconst.tile([128, T], BF16)
        sin_k = pconst.tile([128, T], BF16)
        nc.sync.dma_start(cos_k[:], d_ck[:])
        nc.sync.dma_start(sin_k[:], d_sk[:])
        for c in range(4):
            nc.sync.dma_start(wv[:, c], d_wv[c * 128:(c + 1) * 128, :])
            nc.sync.dma_start(wout[:, c], d_wout[c * 128:(c + 1) * 128, :])
            nc.sync.dma_start(w1[:, c], d_w1[c * 128:(c + 1) * 128, :])
        for c in range(16):
            nc.sync.dma_start(w2[:, c], d_w2[c * 128:(c + 1) * 128, :])
        nc.sync.dma_start(b1t[:], d_b1[:, :])

        ones_bf = pconst.tile([128, 128], BF16)
        nc.gpsimd.memset(ones_bf[:], 1.0)
        inv_d = pconst.tile([128, 1], BF16)
        nc.gpsimd.memset(inv_d[:], 1.0 / D)
        eps_t = pconst.tile([128, 1], F32)
        nc.gpsimd.memset(eps_t[:], EPS)

        # persistent activations
        nT8 = pact.tile([128, 4, T], F8)           # LN1(x)^T fp8 (q/k GEMMs)
        nTb = None if V_FP8 else pact.tile([128, 4, T], BF16)  # for v GEMM
        kT = pact.tile([128, 4, T], BF16)          # k^T (RoPE'd, x WS)
        qT = pact.tile([128, 4, TQ], BF16)         # q^T (RoPE'd, x WS)
        # v natural, per-head blocks of 65 cols: 64 v dims + a trailing ones
        # column so the PV matmul also produces the softmax rowsum (row 64)
        vnat = pact.tile([128, 16, 8, 66], F8)
        nc.gpsimd.memset(vnat[:, :, :, 64:65], 1.0)
        nc.gpsimd.memset(vnat[:, :, :, 65:66], 0.0)
        attnT = pact.tile([128, 4, TQ], BF16)      # normalized attn out ^T
        r1 = pact.tile([128, 4, TQ], F32)          # x + attn_out  (^T)
        r1b = pact.tile([128, 4, TQ], BF16)        # bf16 copy for LN2 stats
        h2 = pact.tile([128, 4, TQ], h2_dt)        # LN2 out

        def ln_transposed(src_bf, n_chunks, ps_stat, ps_bc, pool, apply_srcs,
                          outs):
            """LayerNorm over the partition (d_model) axis of a transposed
            activation. src_bf: [128, 4, n_chunks*512] bf16 tile for stats.
            apply_srcs: per-c list of tiles to read in the apply step (f32 or
            bf16); outs: list of (tile, ) destinations written as
            (apply_src - mu) * rstd."""
            for i in range(n_chunks):
                cs = slice(i * 512, (i + 1) * 512)
                ps_mu = ps_stat.tile([1, 512], F32, tag="ps_mu")
                for c in range(4):
                    nc.tensor.matmul(ps_mu[:], inv_d[:, 0:1], src_bf[:, c, cs],
                                     start=(c == 0), stop=(c == 3))
                sq = pool.tile([128, 4, 512], BF16, tag="ln_sq")
                for c in range(4):
                    nc.vector.tensor_mul(sq[:, c], src_bf[:, c, cs],
                                         src_bf[:, c, cs])
                ps_sq = ps_stat.tile([1, 512], F32, tag="ps_sq")
                for c in range(4):
                    nc.tensor.matmul(ps_sq[:], inv_d[:, 0:1], sq[:, c],
                                     start=(c == 0), stop=(c == 3))
                mu_r = pool.tile([1, 512], BF16, tag="mu_r")
                nc.vector.tensor_copy(mu_r[:], ps_mu[:])
                ex2_r = pool.tile([1, 512], F32, tag="ex2_r")
                nc.vector.tensor_copy(ex2_r[:], ps_sq[:])
                # var = E[x^2] - mu^2 on the narrow stat rows
                m2_r = pool.tile([1, 512], F32, tag="m2_r")
                nc.vector.tensor_mul(m2_r[:], mu_r[:], mu_r[:])
                var_r = pool.tile([1, 512], BF16, tag="var_r")
                nc.vector.tensor_sub(var_r[:], ex2_r[:], m2_r[:])
                bc = ps_bc.tile([128, 2, 512], F32, tag="ps_bc")
                nc.tensor.matmul(bc[:, 0], ones_bf[0:1, :], mu_r[:],
                                 start=True, stop=True)
                nc.tensor.matmul(bc[:, 1], ones_bf[0:1, :], var_r[:],
                                 start=True, stop=True)
                std = pool.tile([128, 512], F32, tag="ln_std")
                nc.scalar.activation(std[:], bc[:, 1], AF.Sqrt, bias=eps_t[:],
                                     scale=1.0)
                brs = pool.tile([128, 512], F32, tag="ln_brs")
                nc.vector.reciprocal(brs[:], std[:])
                for c in range(4):
                    t_sub = pool.tile([128, 512], BF16, tag="ln_tsub")
                    nc.vector.tensor_sub(t_sub[:], apply_srcs[c][:, cs],
                                         bc[:, 0])
                    for out_t in outs:
                        nc.vector.tensor_mul(out_t[:, c, cs], t_sub[:], brs[:])

        # ============ phase A: LN1 + QKV + RoPE ============
        with tc.tile_pool(name="phA", bufs=2) as pa, \
             tc.tile_pool(name="phArope", bufs=2) as pr, \
             tc.tile_pool(name="psumSt", bufs=1, space="PSUM") as ps_stat, \
             tc.tile_pool(name="psumBc", bufs=2, space="PSUM") as ps_bc, \
             tc.tile_pool(name="psumA", bufs=2, space="PSUM") as psum:
            ln_transposed(xt, T // 512, ps_stat, ps_bc, pa,
                          [xt[:, c] for c in range(4)],
                          [nT8] if V_FP8 else [nT8, nTb])

            # q^T [j,t] over the core's half (fp8 DoubleRow)
            for jb in range(4):
                for tcn in range(TQ // 512):
                    ps = psum.tile([128, 512], F32, tag="ps_mm")
                    for c2 in range(2):
                        nc.tensor.matmul(
                            ps[:],
                            wqk[:, 2 * c2:2 * c2 + 2, jb * 128:(jb + 1) * 128],
                            nT8[:, 2 * c2:2 * c2 + 2, tcn * 512:(tcn + 1) * 512],
                            start=(c2 == 0), stop=(c2 == 1), perf_mode=DR)
                    nc.scalar.copy(qT[:, jb, tcn * 512:(tcn + 1) * 512], ps[:])
            # k^T [j,t] over full T (fp8 DoubleRow)
            for jb in range(4):
                for tcn in range(T // 512):
                    ps = psum.tile([128, 512], F32, tag="ps_mm")
                    for c2 in range(2):
                        nc.tensor.matmul(
                            ps[:],
                            wqk[:, 2 * c2:2 * c2 + 2,
                                512 + jb * 128:512 + (jb + 1) * 128],
                            nT8[:, 2 * c2:2 * c2 + 2, tcn * 512:(tcn + 1) * 512],
                            start=(c2 == 0), stop=(c2 == 1), perf_mode=DR)
                    nc.scalar.copy(kT[:, jb, tcn * 512:(tcn + 1) * 512], ps[:])
            # v natural [t, dv] over full T (lhsT = LN1(x)^T chunks)
            for tb in range(T // 128):
                ps = psum.tile([128, 8, 64], F32, tag="ps_mm")
                if V_FP8:
                    for c2 in range(2):
                        nc.tensor.matmul(
                            ps[:],
                            nT8[:, 2 * c2:2 * c2 + 2, tb * 128:(tb + 1) * 128],
                            wv[:, 2 * c2:2 * c2 + 2, :],
                            start=(c2 == 0), stop=(c2 == 1), perf_mode=DR)
                    nc.scalar.mul(vnat[:, tb, :, 0:64], ps[:], 1.0 / WS)
                else:
                    for c in range(4):
                        nc.tensor.matmul(
                            ps[:], nTb[:, c, tb * 128:(tb + 1) * 128],
                            wv[:, c, :], start=(c == 0), stop=(c == 3))
                    nc.scalar.copy(vnat[:, tb, :, 0:64], ps[:])

            # ---------------- RoPE on q^T and k^T ----------------
            def rope(tsb, jb, t0, cos_t, sin_t, coff):
                W = 512
                view = tsb[:, jb, t0:t0 + W]
                cs = slice(coff + t0, coff + t0 + W)
                qsw = pr.tile([128, W], BF16, tag="rope_swap")
                nc.sync.dma_start(qsw[0:32], view[32:64])
                nc.sync.dma_start(qsw[32:64], view[0:32])
                nc.sync.dma_start(qsw[64:96], view[96:128])
                nc.sync.dma_start(qsw[96:128], view[64:96])
                m1 = pr.tile([128, W], BF16, tag="rope_m1")
                nc.vector.tensor_mul(m1[:], view, cos_t[:, cs])
                m2 = pr.tile([128, W], BF16, tag="rope_m2")
                nc.vector.tensor_mul(m2[:], qsw[:], sin_t[:, cs])
                nc.vector.tensor_add(view, m1[:], m2[:])

            for jb in range(4):
                for t0 in range(0, TQ, 512):
                    rope(qT, jb, t0, cos_k, sin_k, 0)
                for t0 in range(0, T, 512):
                    rope(kT, jb, t0, cos_k, sin_k, 0)

        # ============ phase B: attention ============
        with tc.tile_pool(name="phB", bufs=2) as pb, \
             tc.tile_pool(name="phBo", bufs=9) as po_pool, \
             tc.tile_pool(name="phBe", bufs=4) as pbe, \
             tc.tile_pool(name="psumS", bufs=2, space="PSUM") as ps_s, \
             tc.tile_pool(name="psumPV", bufs=2, space="PSUM") as ps_pv, \
             tc.tile_pool(name="psumBc2", bufs=2, space="PSUM") as ps_bc2:
            for qc in range(TQ // 512):
                qs = slice(qc * 512, (qc + 1) * 512)
                # rowsums of heads 0-3 / 4-7 land on partitions {0,32,64,96}
                rsA = pb.tile([128, 512], BF16, tag="rsA")
                rsB = pb.tile([128, 512], BF16, tag="rsB")
                o_ts = []
                for h in range(H):
                    g, par = h // 2, h % 2
                    po = par * 64      # partition offset of this head's rows
                    pv = ps_pv.tile([128, 512], F32, tag="ps_pv")
                    for pair in range(T // 256):
                        ps2 = ps_s.tile([128, 2, 512], F32, tag="ps_score")
                        for hf in range(2):
                            kb = 2 * pair + hf
                            nc.tensor.matmul(
                                ps2[:, hf],
                                kT[po:po + 64, g, kb * 128:(kb + 1) * 128],
                                qT[po:po + 64, g, qs],
                                start=True, stop=True)
                        e2 = pbe.tile([128, 2, 512], F8, tag="E2")
                        nc.scalar.activation(e2[:], ps2[:], AF.Exp,
                                             scale=0.125 / (WS * WS))
                        # O^T (rows 0..63) + softmax rowsum (row 64)
                        nc.tensor.matmul(
                            pv[0:66],
                            vnat[:, 2 * pair:2 * pair + 2, h, :],
                            e2[:],
                            start=(pair == 0), stop=(pair == T // 256 - 1),
                            perf_mode=DR)
                    o_t = po_pool.tile([128, 512], BF16, tag="o_t")
                    nc.scalar.copy(o_t[0:65], pv[0:65])
                    rs_t = rsA if h < 4 else rsB
                    rp = 32 * (h % 4)
                    nc.sync.dma_start(rs_t[rp:rp + 1], o_t[64:65])
                    o_ts.append(o_t)
                # batched reciprocal of all 8 rowsums (2 ops), then per-head
                # rank-1 broadcast + normalize
                riA = pb.tile([128, 512], BF16, tag="riA")
                riB = pb.tile([128, 512], BF16, tag="riB")
                with nc.allow_low_precision(
                        reason="softmax rowsum reciprocal tolerates bf16"):
                    nc.vector.reciprocal(riA[:], rsA[:])
                    nc.vector.reciprocal(riB[:], rsB[:])
                for h in range(H):
                    g, par = h // 2, h % 2
                    po = par * 64
                    ri_t = riA if h < 4 else riB
                    rp = 32 * (h % 4)
                    bc = ps_bc2.tile([128, 512], F32, tag="ps_bc")
                    nc.tensor.matmul(
                        bc[0:64],
                        ones_bf[rp:rp + 1, 0:64],
                        ri_t[rp:rp + 1, :],
                        start=True, stop=True, tile_position=(rp, 0))
                    # partition-shifted write is legal (out base 64, 64 rows)
                    nc.vector.tensor_mul(attnT[po:po + 64, g, qs],
                                         o_ts[h][0:64], bc[0:64])

        # ============ phase C: out-proj + residual + LN2 ============
        with tc.tile_pool(name="phC", bufs=2) as pc, \
             tc.tile_pool(name="psumSt2", bufs=1, space="PSUM") as ps_stat2, \
             tc.tile_pool(name="psumBc3", bufs=2, space="PSUM") as ps_bc3, \
             tc.tile_pool(name="psumC", bufs=2, space="PSUM") as psum:
            for db in range(4):
                for qc in range(TQ // 512):
                    qs = slice(qc * 512, (qc + 1) * 512)
                    ps = psum.tile([128, 512], F32, tag="ps_mm")
                    for c in range(4):
                        nc.tensor.matmul(
                            ps[:], wout[:, c, db * 128:(db + 1) * 128],
                            attnT[:, c, qs],
                            start=(c == 0), stop=(c == 3))
                    nc.vector.tensor_add(r1[:, db, qs], ps[:], xt[:, db, qs])
                    nc.vector.tensor_copy(r1b[:, db, qs], r1[:, db, qs])

            ln_transposed(r1b, TQ // 512, ps_stat2, ps_bc3, pc,
                          [r1[:, c] for c in range(4)], [h2])

        # ============ phase D: MLP ============
        with tc.tile_pool(name="phD", bufs=2) as pd, \
             tc.tile_pool(name="phDa", bufs=1) as pda, \
             tc.tile_pool(name="psumD", bufs=2, space="PSUM") as psum:
            gelu_scale = 1.0 / WS if FC1_FP8 else 1.0
            fc2_scale = 1.0 / WS if FC2_FP8 else 1.0
            for qc in range(TQ // 512):
                qs = slice(qc * 512, (qc + 1) * 512)
                a_t = pda.tile([128, 16, 512], a_dt, tag="a_t")
                for jb in range(16):
                    ps = psum.tile([128, 512], F32, tag="ps_mm")
                    if FC1_FP8:
                        for c2 in range(2):
                            nc.tensor.matmul(
                                ps[:],
                                w1[:, 2 * c2:2 * c2 + 2,
                                   jb * 128:(jb + 1) * 128],
                                h2[:, 2 * c2:2 * c2 + 2, qs],
                                start=(c2 == 0), stop=(c2 == 1), perf_mode=DR)
                    else:
                        for c in range(4):
                            nc.tensor.matmul(
                                ps[:], w1[:, c, jb * 128:(jb + 1) * 128],
                                h2[:, c, qs], start=(c == 0), stop=(c == 3))
                    nc.scalar.activation(a_t[:, jb], ps[:], AF.Gelu,
                                         bias=b1t[:, jb:jb + 1],
                                         scale=gelu_scale)
                for db in range(4):
                    ps = psum.tile([128, 512], F32, tag="ps_mm")
                    if FC2_FP8:
                        for j2 in range(8):
                            nc.tensor.matmul(
                                ps[:],
                                w2[:, 2 * j2:2 * j2 + 2,
                                   db * 128:(db + 1) * 128],
                                a_t[:, 2 * j2:2 * j2 + 2],
                                start=(j2 == 0), stop=(j2 == 7), perf_mode=DR)
                    else:
                        for jb in range(16):
                            nc.tensor.matmul(
                                ps[:], w2[:, jb, db * 128:(db + 1) * 128],
                                a_t[:, jb], start=(jb == 0), stop=(jb == 15))
                    outT = pd.tile([128, 512], F32, tag="outT")
                    nc.vector.scalar_tensor_tensor(
                        outT[:], ps[:], fc2_scale, r1[:, db, qs],
                        op0=ALU.mult, op1=ALU.add)
                    nc.sync.dma_start(
                        d_out[db * 128:(db + 1) * 128, qs], outT[:])

        if DEBUG:
            for name, t in [("nT8", nT8), ("nTb", nTb), ("qT", qT),
                            ("kT", kT), ("vnat", vnat), ("attnT", attnT),
                            ("r1", r1), ("h2", h2)]:
                nc.sync.dma_start(d_dbg[name][:], t[:])
    nc.finalize()
    return nc


def kernel(x, ln1_g, ln1_b, w_qkv, w_out, ln2_g, ln2_b, w1, b1, w2, b2):
    x = np.asarray(x, np.float32)
    ln1_g = np.asarray(ln1_g, np.float32); ln1_b = np.asarray(ln1_b, np.float32)
    ln2_g = np.asarray(ln2_g, np.float32); ln2_b = np.asarray(ln2_b, np.float32)
    w_qkv = np.asarray(w_qkv, np.float32); w_out = np.asarray(w_out, np.float32)
    w1 = np.asarray(w1, np.float32); b1 = np.asarray(b1, np.float32)
    w2 = np.asarray(w2, np.float32); b2 = np.asarray(b2, np.float32)

    assert not np.any(ln1_b), "nonzero ln1_b not supported by this kernel"

    # exact-math folds: LN affine params into the adjacent weight matrices
    wqkv_f = ln1_g[:, None] * w_qkv
    w1_f = ln2_g[:, None] * w1
    b1_f = b1 + ln2_b @ w1

    bf = ml_dtypes.bfloat16
    f8 = ml_dtypes.float8_e4m3

    def to_f8(a, scale):
        return np.ascontiguousarray(
            np.clip(a * scale, -240.0, 240.0).astype(f8))

    crep, srep = _rope_tables()

    common = {
        "wqk": to_f8(wqkv_f[:, :2 * D], WS),
        "wv": (to_f8(wqkv_f[:, 2 * D:], WS) if V_FP8
               else np.ascontiguousarray(wqkv_f[:, 2 * D:].astype(bf))),
        "wout": np.ascontiguousarray(w_out.astype(bf)),
        "w1": (to_f8(w1_f, WS) if FC1_FP8
               else np.ascontiguousarray(w1_f.astype(bf))),
        "w2": (to_f8(w2, WS) if FC2_FP8
               else np.ascontiguousarray(w2.astype(bf))),
        "b1t": np.ascontiguousarray(
            b1_f.reshape(DFF // 128, 128).T.astype(np.float32)),
    }
    in_maps = []
    for c in range(NC):
        b, half = c // 2, c % 2
        t0 = half * TQ
        rot = np.r_[t0:T, 0:t0]
        m = dict(common)
        m["xt"] = np.ascontiguousarray(x[b].T[:, rot].astype(bf))
        m["cos_k"] = np.ascontiguousarray(crep[:, rot])
        m["sin_k"] = np.ascontiguousarray(srep[:, rot])
        in_maps.append(m)

    if "prog" not in _CACHE:
        _CACHE["prog"] = _build_program()
    nc = _CACHE["prog"]

    _CACHE["in_maps"] = in_maps
    res = run_bass_kernel_spmd(nc, in_maps, core_ids=list(range(NC)))
    out = np.empty((B, T, D), np.float32)
    for c in range(NC):
        b, half = c // 2, c % 2
        out[b, half * TQ:(half + 1) * TQ] = res.results[c]["out"].T
    out += b2[None, None, :]
    return out
